# revision 1
# baseline (speedup 1.0000x reference)
"""GCN encoder (2x GCNConv + global max pool + 2x FC) on 8 TRN2 NeuronCores.

Strategy (sharding hint: node partitioning + halo exchange; here graph-aligned
node sharding so pooling is core-local):
  - Nodes are sharded contiguously at *graph* boundaries: rank r owns graphs
    [r*GR, (r+1)*GR) and therefore a contiguous global node range. Each rank's
    slice is padded to a common NMAX (multiple of 128).
  - Edges are routed to the rank owning their dst node. Message passing is a
    pure DMA pipeline: dma_gather (rows of the feature table by src index)
    -> per-edge scale by dinv[src] (layer 1 only; layer-2 table is pre-scaled)
    -> dma_scatter_add into the rank-local aggregation buffer (dst index).
    Self-loop terms are folded in algebraically during the post phase.
  - GCN normalization: norm = dinv[src]*dinv[dst] splits into a per-edge
    src factor and a per-node dst factor applied after aggregation.
    out_l = relu((dinv .* (agg + dinv .* h)) @ W_l + b_l)
  - dma_gather indices are int16, so the feature table is processed in 4
    chunks of <=32767 rows; edges are bucketed host-side by src chunk.
  - After layer 1, h1s = dinv .* h1 is AllGather'd (the halo exchange - the
    graph is random so the halo is essentially everything).
  - Max pooling: h2 is produced feature-major (h2T); per-graph max is a
    fixed number of clamped fixed-width windowed reduce_max ops (overlap is
    idempotent for max) whose start columns are per-core data loaded into
    registers (SPMD-uniform program). Tiny AllGather of pooled partials,
    then the (replicated) FC head runs on every core.

  HW-calibrated constraints baked in below: dma_scatter_add requires unique
  dst indices per instruction (CCE read-modify-write races on duplicates);
  the SWDGE descriptor ring (dynamic_dma_scratch_size/16 entries) must hold
  ~2 descriptors per valid index for the in-flight gather+scatter window,
  hence the per-tile valid cap of 960 with a 4096-entry ring.
"""

import math
from contextlib import ExitStack

import numpy as np

import concourse.bass as bass
import concourse.bacc as bacc
import concourse.mybir as mybir
import concourse.tile as tile
from concourse import library_config

F32 = mybir.dt.float32
I16 = mybir.dt.int16

R = 8          # NeuronCores
C = 4          # src chunks (rank pairs) for int16 gather indices
F = 128        # in dim == hidden
H2 = 256       # 2*hidden
FCD = 512      # fc1 out
PROJ = 128     # fc2 out


# ----------------------------------------------------------------- host prep

def _wrap16(a: np.ndarray) -> np.ndarray:
    """[T] int -> [128, T//16] int16: idx j at (j%16, j//16), replicated to
    all 8 groups of 16 partitions."""
    T = a.shape[0]
    w = np.ascontiguousarray(a.reshape(T // 16, 16).T).astype(np.int16)
    return np.tile(w, (8, 1))


def _msg_layout(a: np.ndarray, te: int) -> np.ndarray:
    """[T] f32 -> [128, T//128]: value j at (j%128, j//128) (gather msg layout)."""
    return np.ascontiguousarray(a.reshape(te // 128, 128).T).astype(np.float32)


def preprocess(x, edge_index, batch, te=8192, win=None):
    x = np.asarray(x, dtype=np.float32)
    src = np.asarray(edge_index[0], dtype=np.int64)
    dst = np.asarray(edge_index[1], dtype=np.int64)
    batch = np.asarray(batch, dtype=np.int64)
    N = x.shape[0]
    G = int(batch.max()) + 1 if batch.size else 1
    G = max(G, R)  # at least one graph per rank
    # graphs are [0, G); harness G=128
    assert G % R == 0, f"graphs {G} not divisible by {R}"
    GR = G // R

    sizes = np.bincount(batch, minlength=G)
    gstart = np.concatenate([[0], np.cumsum(sizes)])  # [G+1]
    rb = gstart[::GR].copy()                          # [R+1] rank node bounds
    assert rb[-1] == N
    Nc = np.diff(rb)
    NMAX = int(math.ceil(max(int(Nc.max()), 128) / 128) * 128)
    assert 2 * NMAX <= 32766, "int16 gather index overflow"
    NTL = NMAX // 128
    AGG_ROWS = NMAX + 128  # +128 dump rows for padding edges

    deg = np.bincount(dst, minlength=N).astype(np.float64) + 1.0
    dinv = (1.0 / np.sqrt(deg)).astype(np.float32)

    # per-edge routing
    rk_dst = np.searchsorted(rb, dst, side="right") - 1
    rk_src = np.searchsorted(rb, src, side="right") - 1
    ch_src = rk_src // 2

    l1rel_all = (src - rb[2 * ch_src]).astype(np.int64)
    l2rel_all = ((rk_src % 2) * NMAX + (src - rb[rk_src])).astype(np.int64)
    dstrel_all = (dst - rb[rk_dst]).astype(np.int64)
    scale_all = dinv[src].astype(np.float32)

    # --- edge -> tile assignment: dma_scatter_add races on duplicate dst
    # indices within one instruction, so each tile must have UNIQUE dsts.
    # The k-th edge of a given (dst, chunk) goes to tile (k + rot(dst)) % T_c.
    cnt = np.zeros((R, C), dtype=np.int64)
    occ_max = np.zeros(C, dtype=np.int64)
    for r in range(R):
        for c in range(C):
            m = (rk_dst == r) & (ch_src == c)
            cnt[r, c] = int(m.sum())
            if cnt[r, c]:
                occ_max[c] = max(occ_max[c],
                                 int(np.bincount(dstrel_all[m]).max()))

    cap = min(te // 2 - 128, 960)  # SWDGE ring: ~2 descs per valid idx
    NT_c = []
    for c in range(C):
        t_need = max(int(math.ceil(cnt[:, c].max() * 1.08 / cap)),
                     int(occ_max[c]), 1)
        while True:  # grow until every (rank, tile) fits with the fake edge
            ok = True
            for r in range(R):
                m = (rk_dst == r) & (ch_src == c)
                ddm = dstrel_all[m]
                if len(ddm) == 0:
                    continue
                order = np.argsort(ddm, kind="stable")
                sorted_d = ddm[order]
                runstart = np.r_[0, np.flatnonzero(np.diff(sorted_d)) + 1]
                occ_sorted = np.arange(len(ddm)) - np.repeat(
                    runstart, np.diff(np.r_[runstart, len(ddm)]))
                occ = np.zeros(len(ddm), dtype=np.int64)
                occ[order] = occ_sorted
                tf = (occ + (ddm * 2654435761 % t_need)) % t_need
                if int(np.bincount(tf, minlength=t_need).max()) + 1 > cap:
                    ok = False
                    break
            if ok:
                break
            t_need += 1
        NT_c.append(t_need)
    NT = sum(NT_c)
    chunk_of = sum(([c] * NT_c[c] for c in range(C)), [])

    es1 = np.full((R, NT, 128, te // 16), -1, dtype=np.int16)
    es2 = np.full((R, NT, 128, te // 16), -1, dtype=np.int16)
    ed = np.full((R, NT, 128, te // 16), -1, dtype=np.int16)
    esc = np.zeros((R, NT, 128, te // 128), dtype=np.float32)
    ecnt = np.zeros((R, NT), dtype=np.int32)

    for r in range(R):
        t0 = 0
        for c in range(C):
            m = (rk_dst == r) & (ch_src == c)
            T_c = NT_c[c]
            l1m = l1rel_all[m]; l2m = l2rel_all[m]
            ddm = dstrel_all[m]; scm = scale_all[m]
            # occurrence index of each edge within its dst group
            order = np.argsort(ddm, kind="stable")
            occ = np.zeros(len(ddm), dtype=np.int64)
            if len(ddm):
                sorted_d = ddm[order]
                runstart = np.r_[0, np.flatnonzero(np.diff(sorted_d)) + 1]
                occ_sorted = np.arange(len(ddm)) - np.repeat(
                    runstart, np.diff(np.r_[runstart, len(ddm)]))
                occ[order] = occ_sorted
            tile_of = (occ + (ddm * 2654435761 % T_c)) % T_c
            # per-tile buffers
            for t in range(T_c):
                sel = tile_of == t
                k = int(sel.sum())
                assert k + 1 <= te // 2 - 64, (
                    f"tile overflow r{r} c{c} t{t}: {k + 1}")
                l1 = np.full(te, -1, dtype=np.int64)
                l2 = np.full(te, -1, dtype=np.int64)
                dd = np.full(te, -1, dtype=np.int64)
                sc = np.zeros(te, dtype=np.float32)
                l1[:k] = l1m[sel]
                l2[:k] = l2m[sel]
                dd[:k] = ddm[sel]
                sc[:k] = scm[sel]
                # one fake edge so every tile has >=1 valid index
                l1[k] = 0
                l2[k] = 0
                dd[k] = NMAX + ((t0 + t) % 128)
                sc[k] = 0.0
                es1[r, t0 + t] = _wrap16(l1)
                es2[r, t0 + t] = _wrap16(l2)
                ed[r, t0 + t] = _wrap16(dd)
                esc[r, t0 + t] = _msg_layout(sc, te)
                ecnt[r, t0 + t] = k + 1
            t0 += T_c

    # per-rank padded node slices
    xc = np.zeros((R, NMAX, F), dtype=np.float32)
    dvc = np.zeros((R, NMAX, 1), dtype=np.float32)
    for r in range(R):
        n = int(Nc[r])
        xc[r, :n] = x[rb[r]:rb[r + 1]]
        dvc[r, :n, 0] = dinv[rb[r]:rb[r + 1]]

    # pooling windows: every graph gets exactly WPG windows of width WIN with
    # data-driven start columns (clamped overlapping windows; idempotent for
    # max). Uniform structure across cores; starts differ per core.
    min_sz = int(sizes.min())
    assert min_sz > 0, "empty graph not supported"
    if win is None:
        win = 512
    win = int(min(win, min_sz))
    wpg = int(math.ceil(int(sizes.max()) / win))
    wstart = np.zeros((R, GR * wpg), dtype=np.int32)
    for r in range(R):
        for j in range(GR):
            g = r * GR + j
            a0 = int(gstart[g] - rb[r])
            sz = int(sizes[g])
            for k in range(wpg):
                wstart[r, j * wpg + k] = min(a0 + k * win, a0 + sz - win)

    cfg = dict(
        N=N, G=G, GR=GR, NMAX=NMAX, NTL=NTL, AGG_ROWS=AGG_ROWS, TE=te,
        NT=NT, chunk_of=chunk_of,
        l1_base=[int(rb[2 * c]) for c in range(C)],
        l1_size=[int(rb[2 * c + 2] - rb[2 * c]) for c in range(C)],
        l2_base=[2 * c * NMAX for c in range(C)],
        l2_size=[2 * NMAX for c in range(C)],
        WIN=win, WPG=wpg,
    )
    per_core = dict(es1=es1, es2=es2, ed=ed, esc=esc, ecnt=ecnt[:, None, :],
                    xc=xc, dvc=dvc,
                    wstart=wstart[:, :, None].astype(np.int32))
    return cfg, per_core


def make_in_maps(cfg, per_core, x, W1, b1, W2, b2, fc1_w, fc1_b, fc2_w, fc2_b):
    x = np.asarray(x, dtype=np.float32)
    w1 = np.asarray(W1, dtype=np.float32)
    b1v = np.asarray(b1, dtype=np.float32).reshape(1, F)
    w2 = np.asarray(W2, dtype=np.float32)
    b2s = np.asarray(b2, dtype=np.float32).reshape(2, 128).T.copy()      # [128,2]
    f1w = np.asarray(fc1_w, dtype=np.float32).reshape(2, 128, FCD)
    f1w = np.ascontiguousarray(f1w.transpose(1, 0, 2)).reshape(128, 2 * FCD)
    f1b = np.asarray(fc1_b, dtype=np.float32).reshape(4, 128).T.copy()   # [128,4]
    f2w = np.asarray(fc2_w, dtype=np.float32).reshape(4, 128, PROJ)
    f2w = np.ascontiguousarray(f2w.transpose(1, 0, 2)).reshape(128, 4 * PROJ)
    f2b = np.asarray(fc2_b, dtype=np.float32).reshape(1, PROJ)
    ident = np.eye(128, dtype=np.float32)

    shared = dict(x=x, w1=w1, b1=b1v, w2=w2, b2s=b2s, f1w=f1w, f1b=f1b,
                  f2w=f2w, f2b=f2b, ident=ident)
    in_maps = []
    for r in range(R):
        m = dict(shared)
        m["xc"] = per_core["xc"][r]
        m["dvc"] = per_core["dvc"][r]
        m["es1"] = per_core["es1"][r]
        m["es2"] = per_core["es2"][r]
        m["ed"] = per_core["ed"][r]
        m["esc"] = per_core["esc"][r]
        m["ecnt"] = per_core["ecnt"][r]
        m["wstart"] = per_core["wstart"][r]
        in_maps.append(m)
    return in_maps


# ------------------------------------------------------------------- builder

def build_program(cfg, debug_outs=False):
    N = cfg["N"]; G = cfg["G"]; GR = cfg["GR"]
    NMAX = cfg["NMAX"]; NTL = cfg["NTL"]; AGG_ROWS = cfg["AGG_ROWS"]
    TE = cfg["TE"]; NT = cfg["NT"]; chunk_of = cfg["chunk_of"]
    WIN = cfg["WIN"]; WPG = cfg["WPG"]
    TE16 = TE // 16; TE128 = TE // 128
    RG = [list(range(R))]

    nc = bacc.Bacc("TRN2", target_bir_lowering=False, debug=False,
                   num_devices=R, dynamic_dma_scratch_size=65536)

    x_d = nc.dram_tensor("x", [N, F], F32, kind="ExternalInput")
    xc_d = nc.dram_tensor("xc", [NMAX, F], F32, kind="ExternalInput")
    dvc_d = nc.dram_tensor("dvc", [NMAX, 1], F32, kind="ExternalInput")
    es1_d = nc.dram_tensor("es1", [NT, 128, TE16], I16, kind="ExternalInput")
    ecnt_d = nc.dram_tensor("ecnt", [1, NT], mybir.dt.int32,
                            kind="ExternalInput")
    es2_d = nc.dram_tensor("es2", [NT, 128, TE16], I16, kind="ExternalInput")
    ed_d = nc.dram_tensor("ed", [NT, 128, TE16], I16, kind="ExternalInput")
    esc_d = nc.dram_tensor("esc", [NT, 128, TE128], F32, kind="ExternalInput")
    w1_d = nc.dram_tensor("w1", [F, F], F32, kind="ExternalInput")
    b1_d = nc.dram_tensor("b1", [1, F], F32, kind="ExternalInput")
    w2_d = nc.dram_tensor("w2", [F, H2], F32, kind="ExternalInput")
    b2s_d = nc.dram_tensor("b2s", [128, 2], F32, kind="ExternalInput")
    f1w_d = nc.dram_tensor("f1w", [128, 2 * FCD], F32, kind="ExternalInput")
    f1b_d = nc.dram_tensor("f1b", [128, 4], F32, kind="ExternalInput")
    f2w_d = nc.dram_tensor("f2w", [128, 4 * PROJ], F32, kind="ExternalInput")
    f2b_d = nc.dram_tensor("f2b", [1, PROJ], F32, kind="ExternalInput")
    ident_d = nc.dram_tensor("ident", [128, 128], F32, kind="ExternalInput")
    wstart_d = nc.dram_tensor("wstart", [GR * WPG, 1], mybir.dt.int32,
                              kind="ExternalInput")

    agg1_d = nc.dram_tensor("agg1", [AGG_ROWS, F], F32)
    agg2_d = nc.dram_tensor("agg2", [AGG_ROWS, F], F32)
    h1s_d = nc.dram_tensor("h1s", [NMAX, F], F32)
    h1full_d = nc.dram_tensor("h1full", [R * NMAX, F], F32, addr_space="Shared")
    h2t_d = nc.dram_tensor("h2t", [2, 128, NMAX], F32)
    gmax_d = nc.dram_tensor("gmax", [2, 128, GR], F32)
    gpool_d = nc.dram_tensor("gpool", [R, 2, 128, GR], F32, addr_space="Shared")
    out_d = nc.dram_tensor("out", [G, PROJ], F32, kind="ExternalOutput")
    dbg = {}
    if debug_outs:
        for nm, shp in [("dbg_agg1", [AGG_ROWS, F]), ("dbg_h1s", [NMAX, F]),
                        ("dbg_agg2", [AGG_ROWS, F]),
                        ("dbg_h2t", [2, 128, NMAX]),
                        ("dbg_gmax", [2, 128, GR]),
                        ("dbg_gpool", [R, 2, 128, GR])]:
            dbg[nm] = nc.dram_tensor(nm, shp, F32, kind="ExternalOutput")

    NEG_INF = float(np.float32(-np.inf))

    with tile.TileContext(nc, num_cores=R) as tc, ExitStack() as stk:
        cp = stk.enter_context(tc.tile_pool(name="consts", bufs=1))
        w1s = cp.tile([F, F], F32)
        b1s = cp.tile([1, F], F32)
        w2s = cp.tile([F, H2], F32)
        b2ss = cp.tile([128, 2], F32)
        f1ws = cp.tile([128, 2 * FCD], F32)
        f1bs = cp.tile([128, 4], F32)
        f2ws = cp.tile([128, 4 * PROJ], F32)
        f2bs = cp.tile([1, PROJ], F32)
        idents = cp.tile([128, 128], F32)
        ones = cp.tile([1, 128], F32)
        ZW = 4096
        zeros = cp.tile([128, ZW], F32)

        nc.gpsimd.load_library(library_config.mlp)
        nc.sync.dma_start(w1s[:], w1_d[:])
        nc.sync.dma_start(b1s[:], b1_d[:])
        nc.sync.dma_start(w2s[:], w2_d[:])
        nc.sync.dma_start(b2ss[:], b2s_d[:])
        nc.sync.dma_start(f1ws[:], f1w_d[:])
        nc.sync.dma_start(f1bs[:], f1b_d[:])
        nc.sync.dma_start(f2ws[:], f2w_d[:])
        nc.sync.dma_start(f2bs[:], f2b_d[:])
        nc.sync.dma_start(idents[:], ident_d[:])
        nc.vector.memset(ones[:], 1.0)
        nc.vector.memset(zeros[:], 0.0)

        # zero both agg buffers
        for agg in (agg1_d, agg2_d):
            flat = agg.rearrange("n f -> (n f)")
            tot = AGG_ROWS * F
            ofs = 0
            while ofs < tot:
                n = min(128 * ZW, tot - ofs)
                assert n % 128 == 0
                nc.sync.dma_start(
                    flat[ofs:ofs + n].rearrange("(p w) -> p w", p=128),
                    zeros[:, : n // 128],
                )
                ofs += n

        # ---- edge pass helper
        ip = stk.enter_context(tc.tile_pool(name="idx", bufs=3))
        mp = stk.enter_context(tc.tile_pool(name="msg", bufs=2))
        ecs = cp.tile([1, NT], mybir.dt.int32)
        nc.sync.dma_start(ecs[:], ecnt_d[:])

        def edge_pass(es_d, table_d, bases, sizes_, agg_d, scale: bool,
                      lbl=""):
            for t in range(NT):
                c = chunk_of[t]
                sidx = ip.tile([128, TE16], I16, tag="esrc")
                nc.sync.dma_start(sidx[:], es_d[t])
                didx = ip.tile([128, TE16], I16, tag="edst")
                nc.sync.dma_start(didx[:], ed_d[t])
                msg = mp.tile([128, TE128, 128], F32, tag="msg")
                if lbl == "" and t < 2:  # init the 2 pool slots once: gather
                    nc.vector.memset(msg[:], 0.0)  # leaves -1-idx slots stale
                tab = table_d[bases[c]:bases[c] + sizes_[c], :]
                with nc.gpsimd.register(f"ec{lbl}{t}") as rg:
                    nc.gpsimd.reg_load(rg, ecs[0:1, t:t + 1])
                    nv = nc.gpsimd.snap(rg)
                    nc.gpsimd.dma_gather(msg[:], tab, sidx[:], TE, nv, F)
                    if scale:
                        sc = ip.tile([128, TE128], F32, tag="esc")
                        nc.sync.dma_start(sc[:], esc_d[t])
                        nc.vector.tensor_tensor(
                            msg[:], msg[:],
                            sc[:].rearrange("p b -> p b ()").broadcast_to(
                                [128, TE128, 128]),
                            mybir.AluOpType.mult,
                        )
                    nc.gpsimd.dma_scatter_add(
                        agg_d[0:AGG_ROWS, :], msg[:], didx[:], TE, nv, F)

        # ---- layer 1 message passing
        edge_pass(es1_d, x_d, cfg["l1_base"], cfg["l1_size"], agg1_d, True)
        tc.strict_bb_all_engine_barrier()
        if debug_outs:
            nc.sync.dma_start(dbg["dbg_agg1"][:, :], agg1_d[:, :])

        # ---- post 1: h1s = dinv*relu((dinv*(agg1 + dinv2*xc)) @ W1 + b1)
        pp = stk.enter_context(tc.tile_pool(name="post", bufs=3))
        tp = stk.enter_context(
            tc.tile_pool(name="tpsum", bufs=2, space="PSUM"))
        mmp = stk.enter_context(
            tc.tile_pool(name="mpsum", bufs=2, space="PSUM"))

        for i in range(NTL):
            sl = slice(i * 128, (i + 1) * 128)
            ag = pp.tile([128, F], F32, tag="ag")
            nc.sync.dma_start(ag[:], agg1_d[sl, :])
            xt = pp.tile([128, F], F32, tag="xt")
            nc.sync.dma_start(xt[:], xc_d[sl, :])
            dv = pp.tile([128, 1], F32, tag="dv")
            nc.sync.dma_start(dv[:], dvc_d[sl, :])
            s = pp.tile([128, F], F32, tag="s")
            nc.vector.scalar_tensor_tensor(
                s[:], xt[:], dv[:], ag[:],
                mybir.AluOpType.mult, mybir.AluOpType.add)
            s2 = pp.tile([128, F], F32, tag="s2")
            nc.vector.tensor_scalar_mul(s2[:], s[:], dv[:])
            pt = tp.tile([128, 128], F32, tag="pt")
            nc.tensor.transpose(pt[:], s2[:], idents[:])
            aT = pp.tile([128, 128], F32, tag="aT")
            nc.vector.tensor_copy(aT[:], pt[:])
            p1 = mmp.tile([128, F], F32, tag="p1")
            nc.tensor.matmul(p1[:], aT[:], w1s[:], start=True, stop=False)
            nc.tensor.matmul(p1[:], ones[:], b1s[:], start=False, stop=True)
            h1 = pp.tile([128, F], F32, tag="h1")
            nc.scalar.activation(h1[:], p1[:],
                                 mybir.ActivationFunctionType.Relu)
            h1s = pp.tile([128, F], F32, tag="h1s")
            nc.vector.tensor_scalar_mul(h1s[:], h1[:], dv[:])
            nc.sync.dma_start(h1s_d[sl, :], h1s[:])

        tc.strict_bb_all_engine_barrier()
        if debug_outs:
            nc.sync.dma_start(dbg["dbg_h1s"][:, :], h1s_d[:, :])
        nc.gpsimd.collective_compute(
            "AllGather", mybir.AluOpType.bypass, replica_groups=RG,
            ins=[h1s_d[:, :]], outs=[h1full_d[:, :]])
        tc.strict_bb_all_engine_barrier()

        # ---- layer 2 message passing (table pre-scaled by dinv)
        edge_pass(es2_d, h1full_d, cfg["l2_base"], cfg["l2_size"], agg2_d, False, lbl="b")
        tc.strict_bb_all_engine_barrier()
        if debug_outs:
            nc.sync.dma_start(dbg["dbg_agg2"][:, :], agg2_d[:, :])

        # ---- post 2: h2T = relu(W2^T @ (dinv*(agg2 + h1s)) + b2), feature-major
        for i in range(NTL):
            sl = slice(i * 128, (i + 1) * 128)
            ag = pp.tile([128, F], F32, tag="ag")
            nc.sync.dma_start(ag[:], agg2_d[sl, :])
            hs = pp.tile([128, F], F32, tag="xt")
            nc.sync.dma_start(hs[:], h1s_d[sl, :])
            dv = pp.tile([128, 1], F32, tag="dv")
            nc.sync.dma_start(dv[:], dvc_d[sl, :])
            s = pp.tile([128, F], F32, tag="s")
            nc.vector.tensor_add(s[:], ag[:], hs[:])
            s2 = pp.tile([128, F], F32, tag="s2")
            nc.vector.tensor_scalar_mul(s2[:], s[:], dv[:])
            pt = tp.tile([128, 128], F32, tag="pt")
            nc.tensor.transpose(pt[:], s2[:], idents[:])
            aT = pp.tile([128, 128], F32, tag="aT")
            nc.vector.tensor_copy(aT[:], pt[:])
            for h in range(2):
                p2 = mmp.tile([128, 128], F32, tag="p1")
                nc.tensor.matmul(p2[:], w2s[:, h * 128:(h + 1) * 128], aT[:],
                                 start=True, stop=True)
                h2t = pp.tile([128, 128], F32, tag="h1")
                nc.scalar.activation(h2t[:], p2[:],
                                     mybir.ActivationFunctionType.Relu,
                                     bias=b2ss[:, h:h + 1])
                nc.sync.dma_start(h2t_d[h, :, sl], h2t[:])

        tc.strict_bb_all_engine_barrier()

        if debug_outs:
            nc.sync.dma_start(dbg["dbg_h2t"][:, :, :], h2t_d[:, :, :])
        # ---- pooling: WPG fixed windows per graph, data-driven start columns
        gm = cp.tile([128, 2 * GR], F32)
        wp = stk.enter_context(tc.tile_pool(name="win", bufs=4))
        wsts = cp.tile([GR * WPG, 1], mybir.dt.int32)
        nc.sync.dma_start(wsts[:], wstart_d[:])
        gslots = cp.tile([128, 2 * GR * WPG], F32)
        for j in range(GR):
            for k in range(WPG):
                w = j * WPG + k
                with nc.gpsimd.register(f"wst{w}") as rg:
                    nc.gpsimd.reg_load(rg, wsts[w:w + 1, 0:1])
                    sv = nc.gpsimd.snap(rg)
                    for h in range(2):
                        wt = wp.tile([128, WIN], F32, tag="wt")
                        nc.gpsimd.dma_start(
                            wt[:], h2t_d[h, :, bass.ds(sv, WIN)])
                        nc.vector.reduce_max(
                            gslots[:, h * GR * WPG + w:h * GR * WPG + w + 1],
                            wt[:], axis=mybir.AxisListType.X)
        for h in range(2):
            for j in range(GR):
                nc.vector.reduce_max(
                    gm[:, h * GR + j:h * GR + j + 1],
                    gslots[:, h * GR * WPG + j * WPG:
                           h * GR * WPG + (j + 1) * WPG],
                    axis=mybir.AxisListType.X)
        nc.sync.dma_start(
            gmax_d[:, :, :].transpose([1, 0, 2]),
            gm[:].rearrange("p (h j) -> p h j", h=2))
        tc.strict_bb_all_engine_barrier()
        nc.gpsimd.collective_compute(
            "AllGather", mybir.AluOpType.bypass, replica_groups=RG,
            ins=[gmax_d[:, :, :]], outs=[gpool_d[:, :, :, :]])
        tc.strict_bb_all_engine_barrier()

        if debug_outs:
            nc.sync.dma_start(dbg["dbg_gmax"][:, :, :], gmax_d[:, :, :])
            nc.sync.dma_start(dbg["dbg_gpool"][:, :, :, :], gpool_d[:, :, :, :])
        # ---- FC head (replicated)
        gts = []
        for h in range(2):
            gt = cp.tile([128, G], F32, tag=f"gt{h}")
            nc.sync.dma_start(
                gt[:].rearrange("p (r j) -> p r j", r=R),
                gpool_d[:, h, :, :].transpose([1, 0, 2]))
            gts.append(gt)
        o1 = []
        for m in range(4):
            pfc = mmp.tile([128, G], F32, tag="p1")
            for h in range(2):
                nc.tensor.matmul(
                    pfc[:], f1ws[:, h * FCD + m * 128: h * FCD + (m + 1) * 128],
                    gts[h][:], start=(h == 0), stop=(h == 1))
            o1m = cp.tile([128, G], F32, tag=f"o1_{m}")
            nc.vector.tensor_scalar_add(o1m[:], pfc[:], f1bs[:, m:m + 1])
            o1.append(o1m)
        pfc2 = mmp.tile([G, PROJ], F32, tag="p1")
        for m in range(4):
            nc.tensor.matmul(pfc2[:], o1[m][:], f2ws[:, m * PROJ:(m + 1) * PROJ],
                             start=(m == 0), stop=False)
        nc.tensor.matmul(pfc2[:], ones[:, :G], f2bs[:], start=False, stop=True)
        osb = cp.tile([G, PROJ], F32)
        nc.vector.tensor_copy(osb[:], pfc2[:])
        nc.sync.dma_start(out_d[:, :], osb[:])

    nc.compile()
    return nc


# -------------------------------------------------------------------- runner

def _timed_spmd(nc, in_maps, iters=3):
    """Mirror bass2jax.run_bass_via_pjrt's multi-core path, but pre-place
    inputs on device and time warm executions (no NTFF hook in this
    container, so wall-clock around the PJRT execute is the HW metric)."""
    import time as _time

    import jax
    from jax.sharding import Mesh, NamedSharding, PartitionSpec
    from jax.experimental.shard_map import shard_map

    from concourse import bass2jax as b2j

    b2j.install_neuronx_cc_hook()
    n_cores = len(in_maps)
    partition_name = (nc.partition_id_tensor.name
                      if nc.partition_id_tensor else None)
    in_names, out_names, out_avals, zero_outs = [], [], [], []
    for alloc in nc.m.functions[0].allocations:
        if not isinstance(alloc, mybir.MemoryLocationSet):
            continue
        name = alloc.memorylocations[0].name
        if alloc.kind == "ExternalInput":
            if name != partition_name:
                in_names.append(name)
        elif alloc.kind == "ExternalOutput":
            shape = tuple(alloc.tensor_shape)
            dtype = mybir.dt.np(alloc.dtype)
            out_names.append(name)
            out_avals.append(jax.core.ShapedArray(shape, dtype))
            zero_outs.append(np.zeros(shape, dtype))
    n_params = len(in_names)
    n_outs = len(out_avals)
    all_in = list(in_names) + list(out_names)
    if partition_name is not None:
        all_in.append(partition_name)
    donate = tuple(range(n_params, n_params + n_outs))

    def _body(*args):
        operands = list(args)
        if partition_name is not None:
            operands.append(b2j.partition_id_tensor())
        outs = b2j._bass_exec_p.bind(
            *operands,
            out_avals=tuple(out_avals),
            in_names=tuple(all_in),
            out_names=tuple(out_names),
            lowering_input_output_aliases=(),
            sim_require_finite=True,
            sim_require_nnan=True,
            nc=nc,
        )
        return tuple(outs)

    devices = jax.devices()[:n_cores]
    mesh = Mesh(np.asarray(devices), ("core",))
    spec = NamedSharding(mesh, PartitionSpec("core"))
    in_specs = (PartitionSpec("core"),) * (n_params + n_outs)
    out_specs = (PartitionSpec("core"),) * n_outs
    sharded = jax.jit(
        shard_map(_body, mesh=mesh, in_specs=in_specs, out_specs=out_specs,
                  check_rep=False),
        donate_argnums=donate, keep_unused=True)

    import sys as _sys
    concat_in = [
        np.concatenate([np.asarray(in_maps[c][k]) for c in range(n_cores)],
                       axis=0)
        for k in in_names
    ]
    tot_mb = sum(a.nbytes for a in concat_in) / 1e6
    print(f"[timed] transferring {tot_mb:.0f} MB inputs", flush=True)
    in_dev = [jax.device_put(a, spec) for a in concat_in]
    jax.block_until_ready(in_dev)
    print("[timed] inputs on device", flush=True)

    times = []
    out_arrs = None
    for _ in range(iters):
        zdev = [jax.device_put(
            np.zeros((n_cores * z.shape[0], *z.shape[1:]), z.dtype), spec)
            for z in zero_outs]
        jax.block_until_ready(zdev)
        t0 = _time.perf_counter()
        out_arrs = sharded(*in_dev, *zdev)
        jax.block_until_ready(out_arrs)
        times.append(_time.perf_counter() - t0)
        print(f"[timed] iter done {times[-1]*1e3:.2f} ms", flush=True)
    results = [
        {name: np.asarray(out_arrs[i]).reshape(n_cores, *out_avals[i].shape)[c]
         for i, name in enumerate(out_names)}
        for c in range(n_cores)
    ]
    return results, times


def kernel(x, edge_index, batch, W1, b1, W2, b2, fc1_w, fc1_b, fc2_w, fc2_b,
           te=2048, _timing=False, _iters=4):
    from concourse.bass_utils import run_bass_kernel_spmd

    cfg, per_core = preprocess(x, edge_index, batch, te=te)
    in_maps = make_in_maps(cfg, per_core, x, W1, b1, W2, b2,
                           fc1_w, fc1_b, fc2_w, fc2_b)
    nc = build_program(cfg)
    if _timing:
        results, times = _timed_spmd(nc, in_maps, iters=_iters)
        return np.asarray(results[0]["out"], dtype=np.float32), times
    res = run_bass_kernel_spmd(nc, in_maps, list(range(R)))
    return np.asarray(res.results[0]["out"], dtype=np.float32)

